# revision 21
# baseline (speedup 1.0000x reference)
"""RBF kernel regression (Gauss transform) on 8 Trainium2 NeuronCores.

Computes out = K @ alpha where K[b, n] = exp(-||z_b - x_n||^2 / 2),
z: [2048, 64], dataset: [100000, 64], alpha: [100000, 16].

Strategy (sharding_hint): shard dataset/alpha row-wise (N) across 8 cores.
Factorize K = exp(z.x) * exp(-x^2/2) * exp(-z^2/2): fold exp(-x^2/2) into
alpha on the host, apply exp(-z^2/2) on the host at the end. Each core then
computes partial[f, b] = sum_n alpha'[n, f] * exp(z.x_n) over its shard.

Per-core device pipeline (operands pre-packed/transposed on host):
  per (tile-pair, b-half) unit:
    cross kt/kb = dsT^T @ zT   (TensorE fp16, row-tiled pair: h0 + h64)
    G = exp(cross)             (ScalarE, no bias -> pure 1024-wide exps)
    AC = alpha'^T @ G          (TensorE bf16, col-tiled pair q0 + q32,
                                single-shot into a transient PSUM slot)
    acc_sb += AC               (VectorE band adds; PSUM slots rotate x4)
"""

import sys

if "/opt/trn_rl_repo" not in sys.path:
    sys.path.insert(0, "/opt/trn_rl_repo")

import numpy as np

B = 2048  # batch (queries)
D = 64  # feature dim
F = 16  # output dim
NCORES = 8
N_FULL = 100000
NS = N_FULL // NCORES  # 12500 rows per core
NT = 98  # n-tiles of 128 rows (12544 padded)
NTH = NT // 2  # 49 tiles per partition-half
NSP = NT * 128  # 12544
HALF_COLS = NTH * 128  # 6272
BHALF = 1024  # b chunk per PSUM slot / ACT instruction
CHUNK_TILES = 7  # dst DMA chunk granularity (7 column-blocks = 896 cols)


def _pack_core_inputs(z, dataset, alpha):
    """Host-side packing: returns (in_maps, w) where w[b] = exp(-0.5*||z_b||^2)."""
    import ml_dtypes

    z = np.ascontiguousarray(z, dtype=np.float32)
    dataset = np.ascontiguousarray(dataset, dtype=np.float32)
    alpha = np.ascontiguousarray(alpha, dtype=np.float32)

    zT = z.T  # [64, B]
    zt_packed = np.concatenate([zT, zT], axis=0).astype(np.float16)  # [128, B]
    z_sq = np.sum(z.astype(np.float64) ** 2, axis=1)
    w = np.exp(-0.5 * z_sq)  # [B], applied on host at the end

    in_maps = []
    for c in range(NCORES):
        ds_c = dataset[c * NS : (c + 1) * NS]
        al_c = alpha[c * NS : (c + 1) * NS]
        dsp = np.zeros((NSP, D), np.float32)
        dsp[:NS] = ds_c
        alp = np.zeros((NSP, F), np.float32)
        alp[:NS] = al_c
        # fold exp(-x^2/2) into alpha (float64 to keep tiny magnitudes exact)
        xsq = np.sum(dsp.astype(np.float64) ** 2, axis=1)
        alp = (alp.astype(np.float64) * np.exp(-0.5 * xsq)[:, None]).astype(
            np.float32
        )

        dsT = dsp.T  # [64, NSP]
        dst_packed = np.concatenate(
            [dsT[:, :HALF_COLS], dsT[:, HALF_COLS:]], axis=0
        ).astype(np.float16)  # [128, 6272]
        # pair layout: cols [32p, 32p+16) = tile p (h0), [32p+16, 32p+32) = tile NTH+p
        a3 = alp.reshape(NT, 128, F).transpose(1, 0, 2)  # [128, NT, F]
        pairs = np.concatenate([a3[:, :NTH], a3[:, NTH:]], axis=2)  # [128, NTH, 2F]
        alp_packed = np.ascontiguousarray(pairs.reshape(128, NT * F)).astype(
            ml_dtypes.bfloat16
        )  # [128, NT*F]

        in_maps.append(
            {
                "zt": np.ascontiguousarray(zt_packed),
                "dst": np.ascontiguousarray(dst_packed),
                "alp": alp_packed,
            }
        )
    return in_maps, w


def build_nc(nt=NT):
    """Build the Bass module. nt can be reduced for simulator smoke tests."""
    import concourse.bass as bass
    import concourse.tile as tile
    from concourse import bacc, mybir

    assert nt % 2 == 0
    nth = nt // 2
    half_cols = nth * 128

    f32 = mybir.dt.float32
    f16 = mybir.dt.float16
    bf16 = mybir.dt.bfloat16

    nc = bacc.Bacc("TRN2", target_bir_lowering=False, debug=False)
    zt_d = nc.dram_tensor("zt", [128, B], f16, kind="ExternalInput").ap()
    dst_d = nc.dram_tensor("dst", [128, half_cols], f16, kind="ExternalInput").ap()
    alp_d = nc.dram_tensor("alp", [128, nt * F], bf16, kind="ExternalInput").ap()
    out_d = nc.dram_tensor("out", [64, B], f32, kind="ExternalOutput").ap()

    chunk_tiles = CHUNK_TILES if nth % CHUNK_TILES == 0 else 1
    n_chunks = nth // chunk_tiles
    chunk_cols = chunk_tiles * 128

    with tile.TileContext(nc) as tc:
        with (
            tc.tile_pool(name="consts", bufs=1) as consts,
            tc.tile_pool(name="g", bufs=6) as gpool,
            tc.tile_pool(name="ps_x", bufs=3, space="PSUM") as ps_x,
            tc.tile_pool(name="ps_acc", bufs=1, space="PSUM") as ps_acc,
        ):
            # First-needed DMAs first: zt0 + dst0 on sync, alp0 on scalar
            # (the second HWDGE ring) so unit 0 unblocks ASAP.
            ac = chunk_tiles * 2 * F  # alpha cols per chunk (pair layout)
            zt_sb = [
                consts.tile([128, 512], f16, tag=f"zt{q}", name=f"ztq{q}")
                for q in range(4)
            ]
            dst_sb = [
                consts.tile([128, chunk_cols], f16, tag=f"dst{j}", name=f"dstc{j}")
                for j in range(n_chunks)
            ]
            alp_sb = [
                consts.tile([128, ac], bf16, tag=f"alp{j}", name=f"alpc{j}")
                for j in range(n_chunks)
            ]
            nc.sync.dma_start(out=zt_sb[0], in_=zt_d[:, 0:512])
            nc.sync.dma_start(out=dst_sb[0], in_=dst_d[:, 0:chunk_cols])
            nc.scalar.dma_start(out=alp_sb[0], in_=alp_d[:, 0:ac])
            for j in range(1, n_chunks):
                nc.sync.dma_start(
                    out=dst_sb[j], in_=dst_d[:, j * chunk_cols : (j + 1) * chunk_cols]
                )
                nc.sync.dma_start(out=alp_sb[j], in_=alp_d[:, j * ac : (j + 1) * ac])
            for q in range(1, 4):
                nc.sync.dma_start(
                    out=zt_sb[q], in_=zt_d[:, q * 512 : (q + 1) * 512]
                )

            out_sb = consts.tile([64, B], f32, tag="out")

            for bq in range(4):
                bs = bq * 512
                acc_t = ps_acc.tile([128, 512], f32, tag="acct", name="acct")
                acc_b = ps_acc.tile([128, 512], f32, tag="accb", name="accb")
                for p in range(nth):
                    chunk = dst_sb[p // chunk_tiles]
                    coff = (p % chunk_tiles) * 128
                    # kt|kb interleaved in one PSUM tile: paired row-tiled MMs
                    x = ps_x.tile([128, BHALF], f32, tag="x", name="x")
                    nc.tensor.matmul(
                        x[:, 0:512],
                        lhsT=chunk[0:64, coff : coff + 128],
                        rhs=zt_sb[bq][0:64, :],
                        start=True,
                        stop=True,
                    )
                    nc.tensor.matmul(
                        x[:, 512:1024],
                        lhsT=chunk[64:128, coff : coff + 128],
                        rhs=zt_sb[bq][64:128, :],
                        start=True,
                        stop=True,
                    )
                    g = gpool.tile([128, BHALF], bf16, tag="g", name="g")
                    nc.scalar.activation(
                        out=g, in_=x, func=mybir.ActivationFunctionType.Exp
                    )
                    # paired col-tiled acc MMs into persistent accumulator
                    nc.tensor.matmul(
                        acc_t[0:F, :],
                        lhsT=alp_sb[p // chunk_tiles][
                            :, (p % chunk_tiles) * 2 * F : (p % chunk_tiles) * 2 * F + F
                        ],
                        rhs=g[:, 0:512],
                        start=(p == 0),
                        stop=(p == nth - 1),
                        tile_position=(0, 0),
                    )
                    nc.tensor.matmul(
                        acc_b[32 : 32 + F, :],
                        lhsT=alp_sb[p // chunk_tiles][
                            :,
                            (p % chunk_tiles) * 2 * F
                            + F : (p % chunk_tiles) * 2 * F
                            + 2 * F,
                        ],
                        rhs=g[:, 512:1024],
                        start=(p == 0),
                        stop=(p == nth - 1),
                        tile_position=(0, 32),
                    )
                nc.vector.tensor_copy(
                    out=out_sb[0:F, bs : bs + 512], in_=acc_t[0:F, :]
                )
                nc.vector.tensor_copy(
                    out=out_sb[32 : 32 + F, bs : bs + 512], in_=acc_b[32 : 32 + F, :]
                )

            nc.sync.dma_start(out=out_d[0:F, :], in_=out_sb[0:F, :])
            nc.sync.dma_start(
                out=out_d[32 : 32 + F, :], in_=out_sb[32 : 32 + F, :]
            )

    nc.compile()
    return nc


_NC_CACHE = []


def run_on_cores(in_maps, trace=False, **kwargs):
    from concourse.bass_utils import run_bass_kernel_spmd

    if not _NC_CACHE:
        _NC_CACHE.append(build_nc())
    return run_bass_kernel_spmd(
        _NC_CACHE[0], in_maps, core_ids=list(range(NCORES)), trace=trace, **kwargs
    )


def kernel(z, dataset, alpha):
    in_maps, w = _pack_core_inputs(z, dataset, alpha)
    res = run_on_cores(in_maps, trace=False)
    total = np.zeros((F, B), np.float64)
    for r in res.results:
        o = r["out"].astype(np.float64)  # [64, B]
        total += o[0:F] + o[32 : 32 + F]
    total *= w[None, :]
    return np.ascontiguousarray(total.T.astype(np.float32))


# revision 22
# speedup vs baseline: 1.0111x; 1.0111x over previous
"""RBF kernel regression (Gauss transform) on 8 Trainium2 NeuronCores.

Computes out = K @ alpha where K[b, n] = exp(-||z_b - x_n||^2 / 2),
z: [2048, 64], dataset: [100000, 64], alpha: [100000, 16].

Strategy (sharding_hint): shard dataset/alpha row-wise (N) across 8 cores.
Factorize K = exp(z.x) * exp(-x^2/2) * exp(-z^2/2): fold exp(-x^2/2) into
alpha on the host, apply exp(-z^2/2) on the host at the end. Each core then
computes partial[f, b] = sum_n alpha'[n, f] * exp(z.x_n) over its shard.

Per-core device pipeline (operands pre-packed/transposed on host):
  per (tile-pair, b-half) unit:
    cross kt/kb = dsT^T @ zT   (TensorE fp16, row-tiled pair: h0 + h64)
    G = exp(cross)             (ScalarE, no bias -> pure 1024-wide exps)
    AC = alpha'^T @ G          (TensorE bf16, col-tiled pair q0 + q32,
                                single-shot into a transient PSUM slot)
    acc_sb += AC               (VectorE band adds; PSUM slots rotate x4)
"""

import sys

if "/opt/trn_rl_repo" not in sys.path:
    sys.path.insert(0, "/opt/trn_rl_repo")

import numpy as np

B = 2048  # batch (queries)
D = 64  # feature dim
F = 16  # output dim
NCORES = 8
N_FULL = 100000
NS = N_FULL // NCORES  # 12500 rows per core
NT = 98  # n-tiles of 128 rows (12544 padded)
NTH = NT // 2  # 49 tiles per partition-half
NSP = NT * 128  # 12544
HALF_COLS = NTH * 128  # 6272
BHALF = 1024  # b chunk per PSUM slot / ACT instruction
CHUNK_TILES = 7  # dst DMA chunk granularity (7 column-blocks = 896 cols)


def _pack_core_inputs(z, dataset, alpha):
    """Host-side packing: returns (in_maps, w) where w[b] = exp(-0.5*||z_b||^2)."""
    import ml_dtypes

    z = np.ascontiguousarray(z, dtype=np.float32)
    dataset = np.ascontiguousarray(dataset, dtype=np.float32)
    alpha = np.ascontiguousarray(alpha, dtype=np.float32)

    zT = z.T  # [64, B]
    zt_packed = np.concatenate([zT, zT], axis=0).astype(np.float16)  # [128, B]
    z_sq = np.sum(z.astype(np.float64) ** 2, axis=1)
    w = np.exp(-0.5 * z_sq)  # [B], applied on host at the end

    in_maps = []
    for c in range(NCORES):
        ds_c = dataset[c * NS : (c + 1) * NS]
        al_c = alpha[c * NS : (c + 1) * NS]
        dsp = np.zeros((NSP, D), np.float32)
        dsp[:NS] = ds_c
        alp = np.zeros((NSP, F), np.float32)
        alp[:NS] = al_c
        # fold exp(-x^2/2) into alpha (float64 to keep tiny magnitudes exact)
        xsq = np.sum(dsp.astype(np.float64) ** 2, axis=1)
        alp = (alp.astype(np.float64) * np.exp(-0.5 * xsq)[:, None]).astype(
            np.float32
        )

        dsT = dsp.T  # [64, NSP]
        dst_packed = np.concatenate(
            [dsT[:, :HALF_COLS], dsT[:, HALF_COLS:]], axis=0
        ).astype(np.float16)  # [128, 6272]
        # pair layout: cols [32p, 32p+16) = tile p (h0), [32p+16, 32p+32) = tile NTH+p
        a3 = alp.reshape(NT, 128, F).transpose(1, 0, 2)  # [128, NT, F]
        pairs = np.concatenate([a3[:, :NTH], a3[:, NTH:]], axis=2)  # [128, NTH, 2F]
        alp_packed = np.ascontiguousarray(pairs.reshape(128, NT * F)).astype(
            ml_dtypes.bfloat16
        )  # [128, NT*F]

        in_maps.append(
            {
                "zt": np.ascontiguousarray(zt_packed),
                "dst": np.ascontiguousarray(dst_packed),
                "alp": alp_packed,
            }
        )
    return in_maps, w


def build_nc(nt=NT):
    """Build the Bass module. nt can be reduced for simulator smoke tests."""
    import concourse.bass as bass
    import concourse.tile as tile
    from concourse import bacc, mybir

    assert nt % 2 == 0
    nth = nt // 2
    half_cols = nth * 128

    f32 = mybir.dt.float32
    f16 = mybir.dt.float16
    bf16 = mybir.dt.bfloat16

    nc = bacc.Bacc("TRN2", target_bir_lowering=False, debug=False)
    zt_d = nc.dram_tensor("zt", [128, B], f16, kind="ExternalInput").ap()
    dst_d = nc.dram_tensor("dst", [128, half_cols], f16, kind="ExternalInput").ap()
    alp_d = nc.dram_tensor("alp", [128, nt * F], bf16, kind="ExternalInput").ap()
    out_d = nc.dram_tensor("out", [64, B], f32, kind="ExternalOutput").ap()

    chunk_tiles = CHUNK_TILES if nth % CHUNK_TILES == 0 else 1
    n_chunks = nth // chunk_tiles
    chunk_cols = chunk_tiles * 128

    with tile.TileContext(nc) as tc:
        with (
            tc.tile_pool(name="consts", bufs=1) as consts,
            tc.tile_pool(name="g", bufs=3) as gpool,
            tc.tile_pool(name="ps_x", bufs=3, space="PSUM") as ps_x,
            tc.tile_pool(name="ps_acc", bufs=1, space="PSUM") as ps_acc,
        ):
            # First-needed DMAs first: zt0 + dst0 on sync, alp0 on scalar
            # (the second HWDGE ring) so unit 0 unblocks ASAP.
            ac = chunk_tiles * 2 * F  # alpha cols per chunk (pair layout)
            zt_sb = [
                consts.tile([128, 512], f16, tag=f"zt{q}", name=f"ztq{q}")
                for q in range(4)
            ]
            dst_sb = [
                consts.tile([128, chunk_cols], f16, tag=f"dst{j}", name=f"dstc{j}")
                for j in range(n_chunks)
            ]
            alp_sb = [
                consts.tile([128, ac], bf16, tag=f"alp{j}", name=f"alpc{j}")
                for j in range(n_chunks)
            ]
            nc.sync.dma_start(out=zt_sb[0], in_=zt_d[:, 0:512])
            nc.sync.dma_start(out=dst_sb[0], in_=dst_d[:, 0:chunk_cols])
            nc.scalar.dma_start(out=alp_sb[0], in_=alp_d[:, 0:ac])
            for j in range(1, n_chunks):
                nc.sync.dma_start(
                    out=dst_sb[j], in_=dst_d[:, j * chunk_cols : (j + 1) * chunk_cols]
                )
                nc.sync.dma_start(out=alp_sb[j], in_=alp_d[:, j * ac : (j + 1) * ac])
            for q in range(1, 4):
                nc.sync.dma_start(
                    out=zt_sb[q], in_=zt_d[:, q * 512 : (q + 1) * 512]
                )

            out_sb = consts.tile([64, B], f32, tag="out")

            for bq in range(4):
                bs = bq * 512
                acc_t = ps_acc.tile([128, 512], f32, tag="acct", name="acct")
                acc_b = ps_acc.tile([128, 512], f32, tag="accb", name="accb")
                for p in range(nth):
                    chunk = dst_sb[p // chunk_tiles]
                    coff = (p % chunk_tiles) * 128
                    # kt|kb interleaved in one PSUM tile: paired row-tiled MMs
                    x = ps_x.tile([128, BHALF], f32, tag="x", name="x")
                    nc.tensor.matmul(
                        x[:, 0:512],
                        lhsT=chunk[0:64, coff : coff + 128],
                        rhs=zt_sb[bq][0:64, :],
                        start=True,
                        stop=True,
                    )
                    nc.tensor.matmul(
                        x[:, 512:1024],
                        lhsT=chunk[64:128, coff : coff + 128],
                        rhs=zt_sb[bq][64:128, :],
                        start=True,
                        stop=True,
                    )
                    g = gpool.tile([128, BHALF], bf16, tag="g", name="g")
                    nc.scalar.activation(
                        out=g, in_=x, func=mybir.ActivationFunctionType.Exp
                    )
                    # paired col-tiled acc MMs into persistent accumulator
                    nc.tensor.matmul(
                        acc_t[0:F, :],
                        lhsT=alp_sb[p // chunk_tiles][
                            :, (p % chunk_tiles) * 2 * F : (p % chunk_tiles) * 2 * F + F
                        ],
                        rhs=g[:, 0:512],
                        start=(p == 0),
                        stop=(p == nth - 1),
                        tile_position=(0, 0),
                    )
                    nc.tensor.matmul(
                        acc_b[32 : 32 + F, :],
                        lhsT=alp_sb[p // chunk_tiles][
                            :,
                            (p % chunk_tiles) * 2 * F
                            + F : (p % chunk_tiles) * 2 * F
                            + 2 * F,
                        ],
                        rhs=g[:, 512:1024],
                        start=(p == 0),
                        stop=(p == nth - 1),
                        tile_position=(0, 32),
                    )
                nc.vector.tensor_copy(
                    out=out_sb[0:F, bs : bs + 512], in_=acc_t[0:F, :]
                )
                nc.vector.tensor_copy(
                    out=out_sb[32 : 32 + F, bs : bs + 512], in_=acc_b[32 : 32 + F, :]
                )

            nc.sync.dma_start(out=out_d[0:F, :], in_=out_sb[0:F, :])
            nc.sync.dma_start(
                out=out_d[32 : 32 + F, :], in_=out_sb[32 : 32 + F, :]
            )

    nc.compile()
    return nc


_NC_CACHE = []


def run_on_cores(in_maps, trace=False, **kwargs):
    from concourse.bass_utils import run_bass_kernel_spmd

    if not _NC_CACHE:
        _NC_CACHE.append(build_nc())
    return run_bass_kernel_spmd(
        _NC_CACHE[0], in_maps, core_ids=list(range(NCORES)), trace=trace, **kwargs
    )


def kernel(z, dataset, alpha):
    in_maps, w = _pack_core_inputs(z, dataset, alpha)
    res = run_on_cores(in_maps, trace=False)
    total = np.zeros((F, B), np.float64)
    for r in res.results:
        o = r["out"].astype(np.float64)  # [64, B]
        total += o[0:F] + o[32 : 32 + F]
    total *= w[None, :]
    return np.ascontiguousarray(total.T.astype(np.float32))
